# revision 11
# baseline (speedup 1.0000x reference)
"""2-layer GCN (GCNConv x2) on 8 Trainium2 NeuronCores.

Sharding: nodes (rows of x / output) sharded across 8 cores; edges
partitioned by destination core (host-side bucketing).

Math: the GCN symmetric norm dinv[src]*dinv[dst] is separable and matmul
commutes with the (linear) neighborhood sum, so with
    t1 = dinv*(x @ W1)            (per-node, 16 wide)
    out1 = dinv*(S1 + t1) + b1,   S1[d] = sum_{e:dst=d} t1[src_e]
    t2 = dinv*relu(out1)          (16 wide)
    out2 = (dinv*(S2 + t2)) @ W2 + b2
both layers only aggregate 16-wide tables - no per-edge multiplies and no
64-wide gather traffic.

Aggregation (per core): nodes are sorted by in-degree on the host, so
"round r" (edges 8r..8r+8 of each node) covers a dense prefix of nodes.
Slots are gathered with the dma_gather (Ant) instruction from a replicated
DRAM table viewed as 256B quads (4 rows); int16 quad ids fit. A per-slot
one-hot mask (host-built) selects the right row inside the quad on DVE,
then a tree of strided adds reduces the 8 slots per node and accumulates.
Tables are replicated across cores via AllGather of per-core shards; table
rows live in each owner's degree-sorted order and the host translates all
gather indices accordingly, so shard writes are dense.
"""

import math

import numpy as np

import concourse.bass as bass
import concourse.bacc as bacc
import concourse.mybir as mybir
import concourse.tile as tile
from concourse.masks import make_identity

P = 128
KS = 8          # edge slots per node per round
TGMAX = 16      # node t-groups (128 nodes) per gather instruction


def _host_prep(x, edge_index, dinv, n_cores):
    N, IN_CH = x.shape
    NPC = N // n_cores
    SH = NPC + 1                       # shard rows (+1 zero row)
    NPAD = ((NPC + P - 1) // P) * P
    TG = NPAD // P

    src = edge_index[0].astype(np.int64)
    dst = edge_index[1].astype(np.int64)
    core_of_dst = dst // NPC

    cores = []
    for m in range(n_cores):
        sel = np.nonzero(core_of_dst == m)[0]
        d_loc = (dst[sel] - m * NPC).astype(np.int64)
        s_glob = src[sel]
        deg = np.bincount(d_loc, minlength=NPC)
        ord_e = np.argsort(d_loc, kind="stable")
        csr_src = s_glob[ord_e]        # global srcs grouped by local dst
        indptr = np.zeros(NPC + 1, np.int64)
        indptr[1:] = np.cumsum(deg)
        order = np.argsort(-deg, kind="stable")
        cores.append(dict(deg=deg, order=order, deg_sorted=deg[order],
                          csr_src=csr_src, indptr=indptr))

    # global node id -> table row (owner's sorted position within its shard)
    pos_in_owner = np.empty(N, np.int64)
    for m in range(n_cores):
        inv = np.empty(NPC, np.int64)
        inv[cores[m]["order"]] = np.arange(NPC)
        pos_in_owner[m * NPC:(m + 1) * NPC] = inv
    tbl_row = (dinv_idx := None) or ((np.arange(N) // NPC) * SH + pos_in_owner)

    # uniform rounds across cores
    gmax = int(math.ceil(max(c["deg_sorted"][0] if NPC else 0
                             for c in cores) / KS)) or 1
    rounds = []
    for r in range(gmax):
        n_r = max(int((c["deg_sorted"] > KS * r).sum()) for c in cores)
        if n_r == 0:
            break
        rounds.append(((n_r + P - 1) // P) * P)

    # schedule entries: (tg offset g0, tg count, entry index)
    schedule = []
    for r, n_r_pad in enumerate(rounds):
        tgr = n_r_pad // P
        for g0 in range(0, tgr, TGMAX):
            schedule.append((r, g0, min(TGMAX, tgr - g0)))
    NID = P * TGMAX * KS                       # idx slots per full entry
    SUBN = 1024                                # idxs per dma_gather (ring cap)
    NSUB = NID // SUBN
    n_entries = len(schedule)

    karr = np.arange(KS)
    per_core = []
    for m in range(n_cores):
        c = cores[m]
        order_pad = np.full(NPAD, NPC, np.int64)
        order_pad[:NPC] = c["order"]
        deg_pad = np.concatenate([c["deg"], np.zeros(1, np.int64)])

        idx_all = np.zeros((n_entries, P, NSUB, SUBN // 16), np.int16)
        msk_all = np.zeros((n_entries, P, TGMAX * KS * 4), np.float32)
        for e, (r, g0, tg_e) in enumerate(schedule):
            pos = g0 * P + np.arange(tg_e * P)
            nat = order_pad[pos]
            dd = deg_pad[nat]
            bse = c["indptr"][nat] + KS * r
            idx2 = bse[:, None] + karr[None, :]
            valid = (KS * r + karr)[None, :] < dd[:, None]
            idx2 = np.minimum(idx2, max(0, len(c["csr_src"]) - 1))
            if len(c["csr_src"]):
                tr = tbl_row[c["csr_src"][idx2]]
            else:
                tr = np.zeros_like(idx2)
            quad = np.where(valid, tr // 4, 0).astype(np.int64)
            sub = (tr % 4).astype(np.int64)
            # slot j = (tg*KS + k)*128 + p for node tg*128+p
            # [pos=(tg,p), k] -> flat j
            q3 = quad.reshape(tg_e, P, KS).transpose(0, 2, 1)  # [tg,k,p]
            jflat = np.zeros(NID, np.int64)
            jflat[:tg_e * KS * P] = q3.reshape(-1)
            for sb in range(NSUB):
                part = jflat[sb * SUBN:(sb + 1) * SUBN]
                idx_all[e, :, sb, :] = np.tile(
                    part.reshape(-1, 16).T.astype(np.int16), (8, 1))
            mk = np.zeros((tg_e * P, KS, 4), np.float32)
            np.put_along_axis(mk, sub[:, :, None], 1.0, axis=2)
            mk *= valid[:, :, None]
            # -> [p, c=(tg*KS+k), s]
            mk = mk.reshape(tg_e, P, KS, 4).transpose(1, 0, 2, 3)
            msk_all[e, :, :tg_e * KS * 4] = mk.reshape(P, -1)

        xs = np.zeros((NPAD, IN_CH), np.float32)
        xs[:NPC] = x[m * NPC + c["order"]]
        dv = np.zeros(NPAD, np.float32)
        dv[:NPC] = dinv[m * NPC:(m + 1) * NPC][c["order"]]
        per_core.append(dict(
            x_shard=xs,
            idx_all=idx_all.reshape(n_entries * P, NSUB * (SUBN // 16)),
            msk_all=msk_all.reshape(n_entries * P, TGMAX * KS * 4),
            dinv_sb=dv.reshape(TG, P).T.copy()))

    meta = dict(NPC=NPC, SH=SH, NPAD=NPAD, TG=TG, NID=NID, SUBN=SUBN,
                NSUB=NSUB, schedule=schedule)
    return per_core, meta


def _build_nc(meta, IN_CH, HID, OUT, n_cores):
    dt = mybir.dt
    f32, i16 = dt.float32, dt.int16
    SH, NPAD, TG, NID = meta["SH"], meta["NPAD"], meta["TG"], meta["NID"]
    SUBN, NSUB = meta["SUBN"], meta["NSUB"]
    NPC = SH - 1
    TROWS = n_cores * SH                      # 100008, divisible by 4
    QUADS = TROWS // 4
    schedule = meta["schedule"]
    n_entries = len(schedule)
    groups = [list(range(n_cores))]
    FULL_T = (NPC // P) * P                   # 12416 rows in the bulk write
    REM = NPC - FULL_T                        # 84
    mult, add = mybir.AluOpType.mult, mybir.AluOpType.add

    nc = bacc.Bacc(num_devices=n_cores, num_swdge_queues=2)
    xsh = nc.declare_dram_parameter("x_shard", [NPAD, IN_CH], f32, isOutput=False)
    idx_d = nc.declare_dram_parameter("idx_all", [n_entries * P, NSUB * (SUBN // 16)], i16, isOutput=False)
    msk_d = nc.declare_dram_parameter("msk_all", [n_entries * P, TGMAX * KS * 4], f32, isOutput=False)
    dinv_d = nc.declare_dram_parameter("dinv_sb", [P, TG], f32, isOutput=False)
    w1_d = nc.declare_dram_parameter("w1", [IN_CH, HID], f32, isOutput=False)
    w2_d = nc.declare_dram_parameter("w2", [HID, OUT], f32, isOutput=False)
    b1_d = nc.declare_dram_parameter("b1r", [P, HID], f32, isOutput=False)
    b2_d = nc.declare_dram_parameter("b2r", [P, OUT], f32, isOutput=False)
    out_d = nc.declare_dram_parameter("out_shard", [SH, OUT], f32, isOutput=True)

    t1_shard = nc.dram_tensor("t1_shard", [SH, HID], f32)
    t2_shard = nc.dram_tensor("t2_shard", [SH, HID], f32)
    tab1 = nc.dram_tensor("tab1", [QUADS, 4 * HID], f32)
    tab2 = nc.dram_tensor("tab2", [QUADS, 4 * HID], f32)

    with tile.TileContext(nc) as tc:
        with tc.tile_pool(name="const", bufs=1) as cp, \
             tc.tile_pool(name="work", bufs=3) as wp, \
             tc.tile_pool(name="gat", bufs=2) as gp, \
             tc.tile_pool(name="accs", bufs=1) as ap_, \
             tc.tile_pool(name="psum", bufs=2, space="PSUM") as pp:

            ident = cp.tile([P, P], f32)
            make_identity(nc, ident[:])
            w1_s = cp.tile([IN_CH, HID], f32)
            nc.sync.dma_start(out=w1_s[:], in_=w1_d[:])
            w2_s = cp.tile([HID, OUT], f32)
            nc.sync.dma_start(out=w2_s[:], in_=w2_d[:])
            b1_s = cp.tile([P, HID], f32)
            nc.sync.dma_start(out=b1_s[:], in_=b1_d[:])
            b2_s = cp.tile([P, OUT], f32)
            nc.sync.dma_start(out=b2_s[:], in_=b2_d[:])
            dinv_s = cp.tile([P, TG], f32)
            nc.sync.dma_start(out=dinv_s[:], in_=dinv_d[:])
            zrow = cp.tile([1, HID], f32)
            nc.vector.memset(zrow[:], 0.0)
            nc.sync.dma_start(out=t1_shard[NPC:NPC + 1, :], in_=zrow[:1, :])
            nc.sync.dma_start(out=t2_shard[NPC:NPC + 1, :], in_=zrow[:1, :])

            t1_loc = ap_.tile([P, TG * HID], f32)
            t2_loc = ap_.tile([P, TG * HID], f32)
            acc1 = ap_.tile([P, TG * HID], f32)
            acc2 = ap_.tile([P, TG * HID], f32)
            out_loc = ap_.tile([P, TG * OUT], f32)
            nc.vector.memset(acc1[:], 0.0)
            nc.vector.memset(acc2[:], 0.0)

            def shard_write(shard, loc, F):
                # dense strided write: shard row tg*128+p <- loc[p, tg*F:...]
                nc.sync.dma_start(
                    out=shard[0:FULL_T, :].rearrange("(t p) f -> p t f", p=P),
                    in_=loc[:].rearrange("p (t f) -> p t f", f=F)[:, :FULL_T // P, :])
                if REM > 0:
                    nc.sync.dma_start(
                        out=shard[FULL_T:NPC, :],
                        in_=loc[0:REM, (FULL_T // P) * F:(FULL_T // P + 1) * F])

            # ---------------- phase A: t1 = dinv*(x @ W1)
            for t in range(TG):
                xt = wp.tile([P, IN_CH], f32, tag="xt")
                nc.sync.dma_start(out=xt[:], in_=xsh[t * P:(t + 1) * P, :])
                xTp = pp.tile([IN_CH, P], f32, tag="xTp")
                nc.tensor.transpose(xTp[:], xt[:], ident[:])
                xTs = wp.tile([IN_CH, P], f32, tag="xTs")
                nc.vector.tensor_copy(out=xTs[:], in_=xTp[:])
                h1p = pp.tile([P, HID], f32, tag="h1p")
                nc.tensor.matmul(h1p[:], lhsT=xTs[:], rhs=w1_s[:],
                                 start=True, stop=True)
                nc.vector.tensor_tensor(
                    out=t1_loc[:, t * HID:(t + 1) * HID], in0=h1p[:],
                    in1=dinv_s[:, t:t + 1].to_broadcast([P, HID]), op=mult)
            shard_write(t1_shard, t1_loc, HID)
            nc.gpsimd.collective_compute(
                "AllGather", mybir.AluOpType.bypass, replica_groups=groups,
                ins=[t1_shard[:]], outs=[tab1[:].rearrange("q (r f) -> (q r) f", f=HID)])

            # ---------------- aggregation loop
            def agg(tab, acc):
                for e, (r, g0, tg_e) in enumerate(schedule):
                    nid = P * tg_e * KS
                    msk = gp.tile([P, TGMAX * KS * 4], f32, tag="msk")
                    nc.sync.dma_start(out=msk[:],
                                      in_=msk_d[e * P:(e + 1) * P, :])
                    idxs = gp.tile([P, NSUB * (SUBN // 16)], i16, tag="idx")
                    nc.sync.dma_start(out=idxs[:],
                                      in_=idx_d[e * P:(e + 1) * P, :])
                    S = gp.tile([P, TGMAX * KS * 4 * HID], f32, tag="slots")
                    for sb in range(NSUB):
                        snid = min(SUBN, nid - sb * SUBN)
                        if snid <= 0:
                            break
                        c0 = sb * (SUBN // P)
                        w16 = SUBN // 16
                        nc.gpsimd.dma_gather(
                            out_ap=S[:, c0 * 4 * HID:
                                     (c0 + snid // P) * 4 * HID].rearrange(
                                "p (c e) -> p c e", e=4 * HID),
                            in_ap=tab[:],
                            idxs_ap=idxs[:, sb * w16:sb * w16 + snid // 16],
                            num_idxs=snid,
                            num_idxs_reg=snid,
                            elem_size=4 * HID,
                            queue_num=agg.qn % 2)
                        agg.qn += 1
                    C = tg_e * KS
                    s4 = S[:].rearrange("p (c s f) -> p c s f", s=4, f=HID)
                    m4 = msk[:].rearrange("p (c s) -> p c s", s=4)
                    # mask-select the row inside each quad
                    nc.vector.tensor_tensor(
                        out=s4[:, :C, :, :], in0=s4[:, :C, :, :],
                        in1=m4[:, :C, :, None].to_broadcast([P, C, 4, HID]),
                        op=mult)
                    nc.vector.tensor_add(s4[:, :C, 0:2, :],
                                         s4[:, :C, 0:2, :],
                                         s4[:, :C, 2:4, :])
                    nc.vector.tensor_add(s4[:, :C, 0, :],
                                         s4[:, :C, 0, :],
                                         s4[:, :C, 1, :])
                    # k-tree over the 8 slots of each node
                    sk = S[:].rearrange("p (t k q) -> p t k q",
                                        k=KS, q=4 * HID)
                    nc.vector.tensor_add(sk[:, :tg_e, 0:4, :HID],
                                         sk[:, :tg_e, 0:4, :HID],
                                         sk[:, :tg_e, 4:8, :HID])
                    nc.vector.tensor_add(sk[:, :tg_e, 0:2, :HID],
                                         sk[:, :tg_e, 0:2, :HID],
                                         sk[:, :tg_e, 2:4, :HID])
                    red = wp.tile([P, TGMAX * HID], f32, tag="red")
                    r3 = red[:].rearrange("p (t f) -> p t f", f=HID)
                    nc.vector.tensor_add(r3[:, :tg_e, :],
                                         sk[:, :tg_e, 0, :HID],
                                         sk[:, :tg_e, 1, :HID])
                    a3 = acc[:].rearrange("p (t f) -> p t f", f=HID)
                    nc.vector.tensor_add(a3[:, g0:g0 + tg_e, :],
                                         a3[:, g0:g0 + tg_e, :],
                                         r3[:, :tg_e, :])

            agg.qn = 0
            agg(tab1, acc1)

            # finalize L1: t2 = dinv*relu(dinv*(acc1+t1)+b1)
            dbH = dinv_s[:, :, None].to_broadcast([P, TG, HID])
            a1_3 = acc1[:].rearrange("p (t f) -> p t f", f=HID)
            t2_3 = t2_loc[:].rearrange("p (t f) -> p t f", f=HID)
            nc.vector.tensor_add(acc1[:], acc1[:], t1_loc[:])
            nc.vector.tensor_tensor(out=a1_3[:], in0=a1_3[:], in1=dbH, op=mult)
            nc.vector.tensor_tensor(
                out=a1_3[:], in0=a1_3[:],
                in1=b1_s[:, None, :].to_broadcast([P, TG, HID]), op=add)
            nc.vector.tensor_relu(out=acc1[:], in_=acc1[:])
            nc.vector.tensor_tensor(out=t2_3[:], in0=a1_3[:], in1=dbH, op=mult)
            shard_write(t2_shard, t2_loc, HID)
            nc.gpsimd.collective_compute(
                "AllGather", mybir.AluOpType.bypass, replica_groups=groups,
                ins=[t2_shard[:]], outs=[tab2[:].rearrange("q (r f) -> (q r) f", f=HID)])

            agg(tab2, acc2)

            # finalize L2: out = (dinv*(acc2+t2)) @ W2 + b2
            a2_3 = acc2[:].rearrange("p (t f) -> p t f", f=HID)
            nc.vector.tensor_add(acc2[:], acc2[:], t2_loc[:])
            nc.vector.tensor_tensor(out=a2_3[:], in0=a2_3[:], in1=dbH, op=mult)
            for t in range(TG):
                uTp = pp.tile([HID, P], f32, tag="uTp")
                nc.tensor.transpose(uTp[:], acc2[:, t * HID:(t + 1) * HID],
                                    ident[:])
                uTs = wp.tile([HID, P], f32, tag="uTs")
                nc.vector.tensor_copy(out=uTs[:], in_=uTp[:])
                zp = pp.tile([P, OUT], f32, tag="zp")
                nc.tensor.matmul(zp[:], lhsT=uTs[:], rhs=w2_s[:],
                                 start=True, stop=True)
                nc.vector.tensor_add(out_loc[:, t * OUT:(t + 1) * OUT],
                                     zp[:], b2_s[:])
            shard_write(out_d, out_loc, OUT)

    nc.finalize()
    return nc


def _run(x, edge_index, W1, b1, W2, b2, n_cores=8, runner=None):
    N, IN_CH = x.shape
    HID, OUT = W1.shape[1], W2.shape[1]
    x = np.asarray(x, np.float32)
    edge_index = np.asarray(edge_index)
    deg = np.bincount(edge_index[1].astype(np.int64), minlength=N)
    dinv = (1.0 / np.sqrt(deg + 1.0)).astype(np.float32)

    per_core, meta = _host_prep(x, edge_index, dinv, n_cores)
    nc = _build_nc(meta, IN_CH, HID, OUT, n_cores)

    common = dict(
        w1=np.asarray(W1, np.float32), w2=np.asarray(W2, np.float32),
        b1r=np.broadcast_to(np.asarray(b1, np.float32), (P, HID)).copy(),
        b2r=np.broadcast_to(np.asarray(b2, np.float32), (P, OUT)).copy())
    in_maps = [dict(**pc, **common) for pc in per_core]

    if runner is not None:
        results, info = runner(nc, in_maps)
    else:
        r = PjrtRunner(nc, n_cores)
        results, info = r.run(in_maps), r

    NPC = meta["NPC"]
    out = np.empty((N, OUT), np.float32)
    for m in range(n_cores):
        # un-permute: out_shard rows are in degree-sorted order positions?
        # no - shard_write stores row tg*128+p = sorted position n; host maps
        # sorted position back to natural local id via the same order array
        out[m * NPC:(m + 1) * NPC] = results[m]["out_shard"][:NPC]
    # undo the degree-sort permutation per core
    deg_l = deg.reshape(n_cores, NPC)
    for m in range(n_cores):
        order = np.argsort(-deg_l[m], kind="stable")
        tmp = out[m * NPC:(m + 1) * NPC].copy()
        out[m * NPC + order] = tmp[:NPC]
    return out, info


def kernel(**inputs) -> np.ndarray:
    return _run(inputs["x"], inputs["edge_index"], inputs["W1"], inputs["b1"],
                inputs["W2"], inputs["b2"], n_cores=8)[0]


def build_full(x, edge_index, W1, b1, W2, b2, n_cores=8):
    N, IN_CH = x.shape
    HID, OUT = W1.shape[1], W2.shape[1]
    x = np.asarray(x, np.float32)
    edge_index = np.asarray(edge_index)
    deg = np.bincount(edge_index[1].astype(np.int64), minlength=N)
    dinv = (1.0 / np.sqrt(deg + 1.0)).astype(np.float32)
    per_core, meta = _host_prep(x, edge_index, dinv, n_cores)
    nc = _build_nc(meta, IN_CH, HID, OUT, n_cores)
    common = dict(
        w1=np.asarray(W1, np.float32), w2=np.asarray(W2, np.float32),
        b1r=np.broadcast_to(np.asarray(b1, np.float32), (P, HID)).copy(),
        b2r=np.broadcast_to(np.asarray(b2, np.float32), (P, OUT)).copy())
    in_maps = [dict(**pc, **common) for pc in per_core]
    meta["deg"] = deg
    return nc, in_maps, meta


def unpermute_output(results, meta, n_cores=8):
    NPC = meta["NPC"]
    deg = meta["deg"].reshape(n_cores, NPC)
    outs = []
    for m in range(n_cores):
        order = np.argsort(-deg[m], kind="stable")
        sh = results[m]["out_shard"][:NPC]
        nat = np.empty_like(sh)
        nat[order] = sh
        outs.append(nat)
    return np.concatenate(outs)


class PjrtRunner:
    """run_bass_via_pjrt with a persistent jitted executable, so repeated
    executions (for wall-clock timing) skip retracing/recompiling."""

    def __init__(self, nc, n_cores):
        import jax
        from jax.experimental.shard_map import shard_map
        from jax.sharding import Mesh, PartitionSpec
        from concourse import bass2jax, mybir as mb

        bass2jax.install_neuronx_cc_hook()
        self.nc = nc
        self.n_cores = n_cores
        partition_name = (nc.partition_id_tensor.name
                          if nc.partition_id_tensor else None)
        in_names, out_names, out_avals, zero_outs = [], [], [], []
        for alloc in nc.m.functions[0].allocations:
            if not isinstance(alloc, mb.MemoryLocationSet):
                continue
            name = alloc.memorylocations[0].name
            if alloc.kind == "ExternalInput":
                if name != partition_name:
                    in_names.append(name)
            elif alloc.kind == "ExternalOutput":
                shape = tuple(alloc.tensor_shape)
                dtype = mb.dt.np(alloc.dtype)
                out_names.append(name)
                out_avals.append(jax.core.ShapedArray(shape, dtype))
                zero_outs.append(np.zeros(shape, dtype))
        self.in_names, self.out_names = in_names, out_names
        self.out_avals, self.zero_outs = out_avals, zero_outs
        n_params, n_outs = len(in_names), len(out_avals)
        self.n_params = n_params
        all_names = in_names + out_names
        if partition_name is not None:
            all_names.append(partition_name)

        def _body(*args):
            operands = list(args)
            if partition_name is not None:
                operands.append(bass2jax.partition_id_tensor())
            outs = bass2jax._bass_exec_p.bind(
                *operands, out_avals=tuple(out_avals),
                in_names=tuple(all_names), out_names=tuple(out_names),
                lowering_input_output_aliases=(),
                sim_require_finite=True, sim_require_nnan=True, nc=nc)
            return tuple(outs)

        devices = jax.devices()[:n_cores]
        self.mesh = Mesh(np.asarray(devices), ("core",))
        donate = tuple(range(n_params, n_params + n_outs))
        self.sharded = jax.jit(
            shard_map(_body, mesh=self.mesh,
                      in_specs=(PartitionSpec("core"),) * (n_params + n_outs),
                      out_specs=(PartitionSpec("core"),) * n_outs,
                      check_rep=False),
            donate_argnums=donate, keep_unused=True)
        self.jax = jax
        self._dev_in = None

    def put_inputs(self, in_maps):
        concat = [np.concatenate([np.asarray(in_maps[c][n])
                                  for c in range(self.n_cores)], axis=0)
                  for n in self.in_names]
        self._dev_in = [self.jax.device_put(a) for a in concat]

    def _fresh_zeros(self):
        return [np.zeros((self.n_cores * z.shape[0], *z.shape[1:]), z.dtype)
                for z in self.zero_outs]

    def execute(self):
        outs = self.sharded(*self._dev_in, *self._fresh_zeros())
        self.jax.block_until_ready(outs)
        return outs

    def run(self, in_maps):
        self.put_inputs(in_maps)
        outs = self.execute()
        return [
            {n: np.asarray(outs[i]).reshape(self.n_cores,
                                            *self.out_avals[i].shape)[c]
             for i, n in enumerate(self.out_names)}
            for c in range(self.n_cores)
        ]

    def bench(self, iters=5):
        import time
        zeros = [self._fresh_zeros() for _ in range(iters)]
        times = []
        for z in zeros:
            t0 = time.perf_counter()
            outs = self.sharded(*self._dev_in, *z)
            self.jax.block_until_ready(outs)
            times.append(time.perf_counter() - t0)
        return times


# revision 12
# speedup vs baseline: 1.2184x; 1.2184x over previous
"""2-layer GCN (GCNConv x2) on 8 Trainium2 NeuronCores.

Sharding: nodes (rows of x / output) sharded across 8 cores; edges
partitioned by destination core (host-side bucketing).

Math: the GCN symmetric norm dinv[src]*dinv[dst] is separable and matmul
commutes with the (linear) neighborhood sum, so with
    t1 = dinv*(x @ W1)            (per-node, 16 wide)
    out1 = dinv*(S1 + t1) + b1,   S1[d] = sum_{e:dst=d} t1[src_e]
    t2 = dinv*relu(out1)          (16 wide)
    out2 = (dinv*(S2 + t2)) @ W2 + b2
both layers only aggregate 16-wide tables - no per-edge multiplies and no
64-wide gather traffic.

Aggregation (per core): nodes are sorted by in-degree on the host, so
"round r" (edges 8r..8r+8 of each node) covers a dense prefix of nodes.
Slots are gathered with the dma_gather (Ant) instruction from a replicated
DRAM table viewed as 256B quads (4 rows); int16 quad ids fit. A per-slot
one-hot mask (host-built) selects the right row inside the quad on DVE,
then a tree of strided adds reduces the 8 slots per node and accumulates.
Tables are replicated across cores via AllGather of per-core shards; table
rows live in each owner's degree-sorted order and the host translates all
gather indices accordingly, so shard writes are dense.
"""

import math

import numpy as np

import concourse.bass as bass
import concourse.bacc as bacc
import concourse.mybir as mybir
import concourse.tile as tile
from concourse.masks import make_identity

P = 128
KS = 8          # edge slots per node per round
TGMAX = 16      # node t-groups (128 nodes) per gather instruction


def _host_prep(x, edge_index, dinv, n_cores):
    N, IN_CH = x.shape
    NPC = N // n_cores
    SH = NPC + 1                       # shard rows (+1 zero row)
    NPAD = ((NPC + P - 1) // P) * P
    TG = NPAD // P

    src = edge_index[0].astype(np.int64)
    dst = edge_index[1].astype(np.int64)
    core_of_dst = dst // NPC

    cores = []
    for m in range(n_cores):
        sel = np.nonzero(core_of_dst == m)[0]
        d_loc = (dst[sel] - m * NPC).astype(np.int64)
        s_glob = src[sel]
        deg = np.bincount(d_loc, minlength=NPC)
        ord_e = np.argsort(d_loc, kind="stable")
        csr_src = s_glob[ord_e]        # global srcs grouped by local dst
        indptr = np.zeros(NPC + 1, np.int64)
        indptr[1:] = np.cumsum(deg)
        order = np.argsort(-deg, kind="stable")
        cores.append(dict(deg=deg, order=order, deg_sorted=deg[order],
                          csr_src=csr_src, indptr=indptr))

    # global node id -> table row (owner's sorted position within its shard)
    pos_in_owner = np.empty(N, np.int64)
    for m in range(n_cores):
        inv = np.empty(NPC, np.int64)
        inv[cores[m]["order"]] = np.arange(NPC)
        pos_in_owner[m * NPC:(m + 1) * NPC] = inv
    tbl_row = (dinv_idx := None) or ((np.arange(N) // NPC) * SH + pos_in_owner)

    # uniform rounds across cores
    gmax = int(math.ceil(max(c["deg_sorted"][0] if NPC else 0
                             for c in cores) / KS)) or 1
    rounds = []
    for r in range(gmax):
        n_r = max(int((c["deg_sorted"] > KS * r).sum()) for c in cores)
        if n_r == 0:
            break
        rounds.append(((n_r + P - 1) // P) * P)

    # schedule entries: (tg offset g0, tg count, entry index)
    schedule = []
    for r, n_r_pad in enumerate(rounds):
        tgr = n_r_pad // P
        for g0 in range(0, tgr, TGMAX):
            schedule.append((r, g0, min(TGMAX, tgr - g0)))
    NID = P * TGMAX * KS                       # idx slots per full entry
    SUBN = 1024                                # idxs per dma_gather (ring cap)
    NSUB = -(-NID // SUBN)
    n_entries = len(schedule)

    karr = np.arange(KS)
    per_core = []
    for m in range(n_cores):
        c = cores[m]
        order_pad = np.full(NPAD, NPC, np.int64)
        order_pad[:NPC] = c["order"]
        deg_pad = np.concatenate([c["deg"], np.zeros(1, np.int64)])

        idx_all = np.zeros((n_entries, P, NSUB, SUBN // 16), np.int16)
        msk_all = np.zeros((n_entries, P, TGMAX * KS * 4), np.float32)
        for e, (r, g0, tg_e) in enumerate(schedule):
            pos = g0 * P + np.arange(tg_e * P)
            nat = order_pad[pos]
            dd = deg_pad[nat]
            bse = c["indptr"][nat] + KS * r
            idx2 = bse[:, None] + karr[None, :]
            valid = (KS * r + karr)[None, :] < dd[:, None]
            idx2 = np.minimum(idx2, max(0, len(c["csr_src"]) - 1))
            if len(c["csr_src"]):
                tr = tbl_row[c["csr_src"][idx2]]
            else:
                tr = np.zeros_like(idx2)
            quad = np.where(valid, tr // 4, 0).astype(np.int64)
            sub = (tr % 4).astype(np.int64)
            # slot j = (tg*KS + k)*128 + p for node tg*128+p
            # [pos=(tg,p), k] -> flat j
            q3 = quad.reshape(tg_e, P, KS).transpose(0, 2, 1)  # [tg,k,p]
            jflat = np.zeros(NID, np.int64)
            jflat[:tg_e * KS * P] = q3.reshape(-1)
            for sb in range(NSUB):
                part = np.zeros(SUBN, np.int64)
                seg = jflat[sb * SUBN:(sb + 1) * SUBN]
                part[:len(seg)] = seg
                idx_all[e, :, sb, :] = np.tile(
                    part.reshape(-1, 16).T.astype(np.int16), (8, 1))
            mk = np.zeros((tg_e * P, KS, 4), np.float32)
            np.put_along_axis(mk, sub[:, :, None], 1.0, axis=2)
            mk *= valid[:, :, None]
            # -> [p, c=(tg*KS+k), s]
            mk = mk.reshape(tg_e, P, KS, 4).transpose(1, 0, 2, 3)
            msk_all[e, :, :tg_e * KS * 4] = mk.reshape(P, -1)

        xs = np.zeros((NPAD, IN_CH), np.float32)
        xs[:NPC] = x[m * NPC + c["order"]]
        dv = np.zeros(NPAD, np.float32)
        dv[:NPC] = dinv[m * NPC:(m + 1) * NPC][c["order"]]
        per_core.append(dict(
            x_shard=xs,
            idx_all=idx_all.reshape(n_entries * P, NSUB * (SUBN // 16)),
            msk_all=msk_all.reshape(n_entries * P, TGMAX * KS * 4),
            dinv_sb=dv.reshape(TG, P).T.copy()))

    meta = dict(NPC=NPC, SH=SH, NPAD=NPAD, TG=TG, NID=NID, SUBN=SUBN,
                NSUB=NSUB, schedule=schedule)
    return per_core, meta


def _build_nc(meta, IN_CH, HID, OUT, n_cores):
    dt = mybir.dt
    f32, i16 = dt.float32, dt.int16
    SH, NPAD, TG, NID = meta["SH"], meta["NPAD"], meta["TG"], meta["NID"]
    SUBN, NSUB = meta["SUBN"], meta["NSUB"]
    NPC = SH - 1
    TROWS = n_cores * SH                      # 100008, divisible by 4
    QUADS = TROWS // 4
    schedule = meta["schedule"]
    n_entries = len(schedule)
    groups = [list(range(n_cores))]
    FULL_T = (NPC // P) * P                   # 12416 rows in the bulk write
    REM = NPC - FULL_T                        # 84
    mult, add = mybir.AluOpType.mult, mybir.AluOpType.add

    nc = bacc.Bacc(num_devices=n_cores, num_swdge_queues=2)
    xsh = nc.declare_dram_parameter("x_shard", [NPAD, IN_CH], f32, isOutput=False)
    idx_d = nc.declare_dram_parameter("idx_all", [n_entries * P, NSUB * (SUBN // 16)], i16, isOutput=False)
    msk_d = nc.declare_dram_parameter("msk_all", [n_entries * P, TGMAX * KS * 4], f32, isOutput=False)
    dinv_d = nc.declare_dram_parameter("dinv_sb", [P, TG], f32, isOutput=False)
    w1_d = nc.declare_dram_parameter("w1", [IN_CH, HID], f32, isOutput=False)
    w2_d = nc.declare_dram_parameter("w2", [HID, OUT], f32, isOutput=False)
    b1_d = nc.declare_dram_parameter("b1r", [P, HID], f32, isOutput=False)
    b2_d = nc.declare_dram_parameter("b2r", [P, OUT], f32, isOutput=False)
    out_d = nc.declare_dram_parameter("out_shard", [SH, OUT], f32, isOutput=True)

    t1_shard = nc.dram_tensor("t1_shard", [SH, HID], f32)
    t2_shard = nc.dram_tensor("t2_shard", [SH, HID], f32)
    tab1 = nc.dram_tensor("tab1", [QUADS, 4 * HID], f32)
    tab2 = nc.dram_tensor("tab2", [QUADS, 4 * HID], f32)

    with tile.TileContext(nc) as tc:
        with tc.tile_pool(name="const", bufs=1) as cp, \
             tc.tile_pool(name="work", bufs=3) as wp, \
             tc.tile_pool(name="gat", bufs=3) as gp, \
             tc.tile_pool(name="accs", bufs=1) as ap_, \
             tc.tile_pool(name="psum", bufs=2, space="PSUM") as pp:

            ident = cp.tile([P, P], f32)
            make_identity(nc, ident[:])
            w1_s = cp.tile([IN_CH, HID], f32)
            nc.sync.dma_start(out=w1_s[:], in_=w1_d[:])
            w2_s = cp.tile([HID, OUT], f32)
            nc.sync.dma_start(out=w2_s[:], in_=w2_d[:])
            b1_s = cp.tile([P, HID], f32)
            nc.sync.dma_start(out=b1_s[:], in_=b1_d[:])
            b2_s = cp.tile([P, OUT], f32)
            nc.sync.dma_start(out=b2_s[:], in_=b2_d[:])
            dinv_s = cp.tile([P, TG], f32)
            nc.sync.dma_start(out=dinv_s[:], in_=dinv_d[:])
            zrow = cp.tile([1, HID], f32)
            nc.vector.memset(zrow[:], 0.0)
            nc.sync.dma_start(out=t1_shard[NPC:NPC + 1, :], in_=zrow[:1, :])
            nc.sync.dma_start(out=t2_shard[NPC:NPC + 1, :], in_=zrow[:1, :])

            t1_loc = ap_.tile([P, TG * HID], f32)
            t2_loc = ap_.tile([P, TG * HID], f32)
            acc1 = ap_.tile([P, TG * HID], f32)
            acc2 = ap_.tile([P, TG * HID], f32)
            out_loc = ap_.tile([P, TG * OUT], f32)
            nc.vector.memset(acc1[:], 0.0)
            nc.vector.memset(acc2[:], 0.0)

            def shard_write(shard, loc, F):
                # dense strided write: shard row tg*128+p <- loc[p, tg*F:...]
                nc.sync.dma_start(
                    out=shard[0:FULL_T, :].rearrange("(t p) f -> p t f", p=P),
                    in_=loc[:].rearrange("p (t f) -> p t f", f=F)[:, :FULL_T // P, :])
                if REM > 0:
                    nc.sync.dma_start(
                        out=shard[FULL_T:NPC, :],
                        in_=loc[0:REM, (FULL_T // P) * F:(FULL_T // P + 1) * F])

            # ---------------- phase A: t1 = dinv*(x @ W1)
            for t in range(TG):
                xt = wp.tile([P, IN_CH], f32, tag="xt")
                nc.sync.dma_start(out=xt[:], in_=xsh[t * P:(t + 1) * P, :])
                xTp = pp.tile([IN_CH, P], f32, tag="xTp")
                nc.tensor.transpose(xTp[:], xt[:], ident[:])
                xTs = wp.tile([IN_CH, P], f32, tag="xTs")
                nc.vector.tensor_copy(out=xTs[:], in_=xTp[:])
                h1p = pp.tile([P, HID], f32, tag="h1p")
                nc.tensor.matmul(h1p[:], lhsT=xTs[:], rhs=w1_s[:],
                                 start=True, stop=True)
                nc.vector.tensor_tensor(
                    out=t1_loc[:, t * HID:(t + 1) * HID], in0=h1p[:],
                    in1=dinv_s[:, t:t + 1].to_broadcast([P, HID]), op=mult)
            shard_write(t1_shard, t1_loc, HID)
            nc.gpsimd.collective_compute(
                "AllGather", mybir.AluOpType.bypass, replica_groups=groups,
                ins=[t1_shard[:]], outs=[tab1[:].rearrange("q (r f) -> (q r) f", f=HID)])

            # ---------------- aggregation loop
            def agg(tab, acc):
                for e, (r, g0, tg_e) in enumerate(schedule):
                    nid = P * tg_e * KS
                    msk = gp.tile([P, TGMAX * KS * 4], f32, tag="msk")
                    nc.sync.dma_start(out=msk[:],
                                      in_=msk_d[e * P:(e + 1) * P, :])
                    idxs = gp.tile([P, NSUB * (SUBN // 16)], i16, tag="idx")
                    nc.sync.dma_start(out=idxs[:],
                                      in_=idx_d[e * P:(e + 1) * P, :])
                    S = gp.tile([P, TGMAX * KS * 4 * HID], f32, tag="slots")
                    for sb in range(NSUB):
                        snid = min(SUBN, nid - sb * SUBN)
                        if snid <= 0:
                            break
                        c0 = sb * (SUBN // P)
                        w16 = SUBN // 16
                        nc.gpsimd.dma_gather(
                            out_ap=S[:, c0 * 4 * HID:
                                     (c0 + snid // P) * 4 * HID].rearrange(
                                "p (c e) -> p c e", e=4 * HID),
                            in_ap=tab[:],
                            idxs_ap=idxs[:, sb * w16:sb * w16 + snid // 16],
                            num_idxs=snid,
                            num_idxs_reg=snid,
                            elem_size=4 * HID,
                            queue_num=agg.qn % 2)
                        agg.qn += 1
                    C = tg_e * KS
                    s4 = S[:].rearrange("p (c s f) -> p c s f", s=4, f=HID)
                    m4 = msk[:].rearrange("p (c s) -> p c s", s=4)
                    # mask-select the row inside each quad
                    nc.vector.tensor_tensor(
                        out=s4[:, :C, :, :], in0=s4[:, :C, :, :],
                        in1=m4[:, :C, :, None].to_broadcast([P, C, 4, HID]),
                        op=mult)
                    nc.vector.tensor_add(s4[:, :C, 0:2, :],
                                         s4[:, :C, 0:2, :],
                                         s4[:, :C, 2:4, :])
                    nc.vector.tensor_add(s4[:, :C, 0, :],
                                         s4[:, :C, 0, :],
                                         s4[:, :C, 1, :])
                    # k-tree over the 8 slots of each node
                    sk = S[:].rearrange("p (t k q) -> p t k q",
                                        k=KS, q=4 * HID)
                    nc.vector.tensor_add(sk[:, :tg_e, 0:4, :HID],
                                         sk[:, :tg_e, 0:4, :HID],
                                         sk[:, :tg_e, 4:8, :HID])
                    nc.vector.tensor_add(sk[:, :tg_e, 0:2, :HID],
                                         sk[:, :tg_e, 0:2, :HID],
                                         sk[:, :tg_e, 2:4, :HID])
                    red = wp.tile([P, TGMAX * HID], f32, tag="red")
                    r3 = red[:].rearrange("p (t f) -> p t f", f=HID)
                    nc.vector.tensor_add(r3[:, :tg_e, :],
                                         sk[:, :tg_e, 0, :HID],
                                         sk[:, :tg_e, 1, :HID])
                    a3 = acc[:].rearrange("p (t f) -> p t f", f=HID)
                    nc.vector.tensor_add(a3[:, g0:g0 + tg_e, :],
                                         a3[:, g0:g0 + tg_e, :],
                                         r3[:, :tg_e, :])

            agg.qn = 0
            agg(tab1, acc1)

            # finalize L1: t2 = dinv*relu(dinv*(acc1+t1)+b1)
            dbH = dinv_s[:, :, None].to_broadcast([P, TG, HID])
            a1_3 = acc1[:].rearrange("p (t f) -> p t f", f=HID)
            t2_3 = t2_loc[:].rearrange("p (t f) -> p t f", f=HID)
            nc.vector.tensor_add(acc1[:], acc1[:], t1_loc[:])
            nc.vector.tensor_tensor(out=a1_3[:], in0=a1_3[:], in1=dbH, op=mult)
            nc.vector.tensor_tensor(
                out=a1_3[:], in0=a1_3[:],
                in1=b1_s[:, None, :].to_broadcast([P, TG, HID]), op=add)
            nc.vector.tensor_relu(out=acc1[:], in_=acc1[:])
            nc.vector.tensor_tensor(out=t2_3[:], in0=a1_3[:], in1=dbH, op=mult)
            shard_write(t2_shard, t2_loc, HID)
            nc.gpsimd.collective_compute(
                "AllGather", mybir.AluOpType.bypass, replica_groups=groups,
                ins=[t2_shard[:]], outs=[tab2[:].rearrange("q (r f) -> (q r) f", f=HID)])

            agg(tab2, acc2)

            # finalize L2: out = (dinv*(acc2+t2)) @ W2 + b2
            a2_3 = acc2[:].rearrange("p (t f) -> p t f", f=HID)
            nc.vector.tensor_add(acc2[:], acc2[:], t2_loc[:])
            nc.vector.tensor_tensor(out=a2_3[:], in0=a2_3[:], in1=dbH, op=mult)
            for t in range(TG):
                uTp = pp.tile([HID, P], f32, tag="uTp")
                nc.tensor.transpose(uTp[:], acc2[:, t * HID:(t + 1) * HID],
                                    ident[:])
                uTs = wp.tile([HID, P], f32, tag="uTs")
                nc.vector.tensor_copy(out=uTs[:], in_=uTp[:])
                zp = pp.tile([P, OUT], f32, tag="zp")
                nc.tensor.matmul(zp[:], lhsT=uTs[:], rhs=w2_s[:],
                                 start=True, stop=True)
                nc.vector.tensor_add(out_loc[:, t * OUT:(t + 1) * OUT],
                                     zp[:], b2_s[:])
            shard_write(out_d, out_loc, OUT)

    nc.finalize()
    return nc


def _run(x, edge_index, W1, b1, W2, b2, n_cores=8, runner=None):
    N, IN_CH = x.shape
    HID, OUT = W1.shape[1], W2.shape[1]
    x = np.asarray(x, np.float32)
    edge_index = np.asarray(edge_index)
    deg = np.bincount(edge_index[1].astype(np.int64), minlength=N)
    dinv = (1.0 / np.sqrt(deg + 1.0)).astype(np.float32)

    per_core, meta = _host_prep(x, edge_index, dinv, n_cores)
    nc = _build_nc(meta, IN_CH, HID, OUT, n_cores)

    common = dict(
        w1=np.asarray(W1, np.float32), w2=np.asarray(W2, np.float32),
        b1r=np.broadcast_to(np.asarray(b1, np.float32), (P, HID)).copy(),
        b2r=np.broadcast_to(np.asarray(b2, np.float32), (P, OUT)).copy())
    in_maps = [dict(**pc, **common) for pc in per_core]

    if runner is not None:
        results, info = runner(nc, in_maps)
    else:
        r = PjrtRunner(nc, n_cores)
        results, info = r.run(in_maps), r

    NPC = meta["NPC"]
    out = np.empty((N, OUT), np.float32)
    for m in range(n_cores):
        # un-permute: out_shard rows are in degree-sorted order positions?
        # no - shard_write stores row tg*128+p = sorted position n; host maps
        # sorted position back to natural local id via the same order array
        out[m * NPC:(m + 1) * NPC] = results[m]["out_shard"][:NPC]
    # undo the degree-sort permutation per core
    deg_l = deg.reshape(n_cores, NPC)
    for m in range(n_cores):
        order = np.argsort(-deg_l[m], kind="stable")
        tmp = out[m * NPC:(m + 1) * NPC].copy()
        out[m * NPC + order] = tmp[:NPC]
    return out, info


def kernel(**inputs) -> np.ndarray:
    return _run(inputs["x"], inputs["edge_index"], inputs["W1"], inputs["b1"],
                inputs["W2"], inputs["b2"], n_cores=8)[0]


def build_full(x, edge_index, W1, b1, W2, b2, n_cores=8):
    N, IN_CH = x.shape
    HID, OUT = W1.shape[1], W2.shape[1]
    x = np.asarray(x, np.float32)
    edge_index = np.asarray(edge_index)
    deg = np.bincount(edge_index[1].astype(np.int64), minlength=N)
    dinv = (1.0 / np.sqrt(deg + 1.0)).astype(np.float32)
    per_core, meta = _host_prep(x, edge_index, dinv, n_cores)
    nc = _build_nc(meta, IN_CH, HID, OUT, n_cores)
    common = dict(
        w1=np.asarray(W1, np.float32), w2=np.asarray(W2, np.float32),
        b1r=np.broadcast_to(np.asarray(b1, np.float32), (P, HID)).copy(),
        b2r=np.broadcast_to(np.asarray(b2, np.float32), (P, OUT)).copy())
    in_maps = [dict(**pc, **common) for pc in per_core]
    meta["deg"] = deg
    return nc, in_maps, meta


def unpermute_output(results, meta, n_cores=8):
    NPC = meta["NPC"]
    deg = meta["deg"].reshape(n_cores, NPC)
    outs = []
    for m in range(n_cores):
        order = np.argsort(-deg[m], kind="stable")
        sh = results[m]["out_shard"][:NPC]
        nat = np.empty_like(sh)
        nat[order] = sh
        outs.append(nat)
    return np.concatenate(outs)


class PjrtRunner:
    """run_bass_via_pjrt with a persistent jitted executable, so repeated
    executions (for wall-clock timing) skip retracing/recompiling."""

    def __init__(self, nc, n_cores):
        import jax
        from jax.experimental.shard_map import shard_map
        from jax.sharding import Mesh, PartitionSpec
        from concourse import bass2jax, mybir as mb

        bass2jax.install_neuronx_cc_hook()
        self.nc = nc
        self.n_cores = n_cores
        partition_name = (nc.partition_id_tensor.name
                          if nc.partition_id_tensor else None)
        in_names, out_names, out_avals, zero_outs = [], [], [], []
        for alloc in nc.m.functions[0].allocations:
            if not isinstance(alloc, mb.MemoryLocationSet):
                continue
            name = alloc.memorylocations[0].name
            if alloc.kind == "ExternalInput":
                if name != partition_name:
                    in_names.append(name)
            elif alloc.kind == "ExternalOutput":
                shape = tuple(alloc.tensor_shape)
                dtype = mb.dt.np(alloc.dtype)
                out_names.append(name)
                out_avals.append(jax.core.ShapedArray(shape, dtype))
                zero_outs.append(np.zeros(shape, dtype))
        self.in_names, self.out_names = in_names, out_names
        self.out_avals, self.zero_outs = out_avals, zero_outs
        n_params, n_outs = len(in_names), len(out_avals)
        self.n_params = n_params
        all_names = in_names + out_names
        if partition_name is not None:
            all_names.append(partition_name)

        def _body(*args):
            operands = list(args)
            if partition_name is not None:
                operands.append(bass2jax.partition_id_tensor())
            outs = bass2jax._bass_exec_p.bind(
                *operands, out_avals=tuple(out_avals),
                in_names=tuple(all_names), out_names=tuple(out_names),
                lowering_input_output_aliases=(),
                sim_require_finite=True, sim_require_nnan=True, nc=nc)
            return tuple(outs)

        devices = jax.devices()[:n_cores]
        self.mesh = Mesh(np.asarray(devices), ("core",))
        donate = tuple(range(n_params, n_params + n_outs))
        self.sharded = jax.jit(
            shard_map(_body, mesh=self.mesh,
                      in_specs=(PartitionSpec("core"),) * (n_params + n_outs),
                      out_specs=(PartitionSpec("core"),) * n_outs,
                      check_rep=False),
            donate_argnums=donate, keep_unused=True)
        self.jax = jax
        self._dev_in = None

    def put_inputs(self, in_maps):
        concat = [np.concatenate([np.asarray(in_maps[c][n])
                                  for c in range(self.n_cores)], axis=0)
                  for n in self.in_names]
        self._dev_in = [self.jax.device_put(a) for a in concat]

    def _fresh_zeros(self):
        return [np.zeros((self.n_cores * z.shape[0], *z.shape[1:]), z.dtype)
                for z in self.zero_outs]

    def execute(self):
        outs = self.sharded(*self._dev_in, *self._fresh_zeros())
        self.jax.block_until_ready(outs)
        return outs

    def run(self, in_maps):
        self.put_inputs(in_maps)
        outs = self.execute()
        return [
            {n: np.asarray(outs[i]).reshape(self.n_cores,
                                            *self.out_avals[i].shape)[c]
             for i, n in enumerate(self.out_names)}
            for c in range(self.n_cores)
        ]

    def bench(self, iters=5):
        import time
        zeros = [self._fresh_zeros() for _ in range(iters)]
        times = []
        for z in zeros:
            t0 = time.perf_counter()
            outs = self.sharded(*self._dev_in, *z)
            self.jax.block_until_ready(outs)
            times.append(time.perf_counter() - t0)
        return times
